# revision 19
# baseline (speedup 1.0000x reference)
"""Trainium2 Bass kernel for nn_DGT_6485400616966 (soft decision tree forward).

Math (forward pass only):
  pred_z = x @ W_pred.T + b_pred                      [B, 1023]
  The straight-through/one-hot structure collapses: the output depends only on
  the argmax leaf of the tree AND layer, which equals a 10-level tree descent
  following sign(pred_z) at visited nodes (left if z >= 0).
  out = softmax(W_or[:, leaf]) ; std = clip(action_stds[:, leaf], -20, 2)

Device algorithm per core (8192 samples, data-parallel over 8 cores):
  1. PE: z = x @ W_pred.T in ONE fp32r (e8m11) pass. Error |dz| <= ~1e-3;
     rows whose descent passes within TAU=2e-3 of a sign flip are flagged via
     a min-|z|-along-path output and recomputed exactly on the host (~1.7% of
     rows, zero observed misses with 2x margin; see work/check_numerics.py).
  2. Eviction PSUM->SBUF per btile: z kept as fp32 (ap_gather needs 4-byte
     elements for d=1), split ACT (scalar copy) / DVE (tensor copy).
  3. Tree descent per chunk of 16 btiles:
     - DVE DP over levels 0..4 (31 nodes) -> visited level-5 column col5.
     - 5 sequential rounds for levels 5..9: GPSIMD ap_gather of z[col] with
       per-core-wrapped indices (out[p, 16t+p%16] = z[p, idx[p,t]]), DVE
       mask-multiply + reduce extraction, sign -> next column.
     - check gather of the level 0..4 path columns; min |z| over the full
       path -> out_m (host flag).
  4. GPSIMD ap_gather table lookups T[class, leaf] with classes replicated on
     partitions (host pre-permutes rows by pi(p)=8*(p%16)+p//16 so outputs
     land in natural order), PE transpose, contiguous DMA out.
"""

import sys

for _p in ("/opt/trn_rl_repo",):
    if _p not in sys.path:
        sys.path.insert(0, _p)

from contextlib import ExitStack

import numpy as np

import concourse.bacc as bacc
import concourse.bass as bass
import concourse.tile as tile
from concourse import mybir
from concourse.bass_utils import run_bass_kernel_spmd

HEIGHT = 10
IN_DIM = 256
OUT_DIM = 16
BATCH = 65536
N_CORES = 8
B_LOC = BATCH // N_CORES          # 8192 samples per core
NT = B_LOC // 128                 # 64 batch tiles of 128 samples
NB = 16                           # btiles per descent chunk
NCH = NT // NB                    # 4 chunks
NODES = 1024                      # col j+1 = tree node j; col 0 = zero pad
TAU = 2e-3                        # host-fixup flag threshold on min path |z|
F32 = mybir.dt.float32
F32R = mybir.dt.float32r
I16 = mybir.dt.int16
A = mybir.AluOpType


def _build(nc, bias_path: bool):
    xTh = nc.dram_tensor("xTh", [IN_DIM, B_LOC], F32R, kind="ExternalInput")
    Wph = nc.dram_tensor("Wph", [IN_DIM, NODES], F32R, kind="ExternalInput")
    Tout = nc.dram_tensor("Tout", [128, NODES], F32, kind="ExternalInput")
    Tstd = nc.dram_tensor("Tstd", [128, NODES], F32, kind="ExternalInput")
    Ident = nc.dram_tensor("Ident", [128, 128], F32, kind="ExternalInput")
    Mask16 = nc.dram_tensor("Mask16", [128, 256], F32, kind="ExternalInput")
    BigM = nc.dram_tensor("BigM", [128, 1280], F32, kind="ExternalInput")
    B1024 = nc.dram_tensor("B1024", [128, NB], I16, kind="ExternalInput")
    BTH = None
    if bias_path:
        BTH = nc.dram_tensor("BTH", [128, NODES], F32, kind="ExternalInput")
    out_o = nc.dram_tensor("out_o", [B_LOC, OUT_DIM], F32, kind="ExternalOutput")
    out_s = nc.dram_tensor("out_s", [B_LOC, OUT_DIM], F32, kind="ExternalOutput")
    out_m = nc.dram_tensor("out_m", [128, NT], F32, kind="ExternalOutput")

    with tile.TileContext(nc) as tc, ExitStack() as ctx:
        consts = ctx.enter_context(tc.tile_pool(name="consts", bufs=1))
        xpool = ctx.enter_context(tc.tile_pool(name="xpool", bufs=2))
        spool = ctx.enter_context(tc.tile_pool(name="spool", bufs=2))
        dpool = ctx.enter_context(tc.tile_pool(name="dpool", bufs=1))
        gpool = ctx.enter_context(tc.tile_pool(name="gpool", bufs=1))
        opool = ctx.enter_context(tc.tile_pool(name="opool", bufs=2))
        zpool = ctx.enter_context(
            tc.tile_pool(name="zpool", bufs=3, space=bass.MemorySpace.PSUM)
        )
        tpool = ctx.enter_context(
            tc.tile_pool(name="tpool", bufs=2, space=bass.MemorySpace.PSUM)
        )

        wh = [
            consts.tile([128, NODES], F32R, tag=f"wh{k}", name=f"wh{k}")
            for k in range(2)
        ]
        for k in range(2):
            nc.sync.dma_start(out=wh[k], in_=Wph[128 * k : 128 * (k + 1), :])

        t_out = consts.tile([128, NODES], F32)
        t_std = consts.tile([128, NODES], F32)
        ident = consts.tile([128, 128], F32)
        mask16 = consts.tile([128, 256], F32)
        bigm = consts.tile([128, 1280], F32)
        b1024 = consts.tile([128, NB], I16)
        bth = consts.tile([128, NODES], F32) if bias_path else None

        def load_late_consts():
            nc.sync.dma_start(out=t_out, in_=Tout[:, :])
            nc.sync.dma_start(out=t_std, in_=Tstd[:, :])
            nc.sync.dma_start(out=ident, in_=Ident[:, :])
            nc.sync.dma_start(out=mask16, in_=Mask16[:, :])
            nc.sync.dma_start(out=bigm, in_=BigM[:, :])
            nc.sync.dma_start(out=b1024, in_=B1024[:, :])
            if bias_path:
                nc.sync.dma_start(out=bth, in_=BTH[:, :])

        r_out = consts.tile([128, NODES], F32)
        r_std = consts.tile([128, NODES], F32)
        mn_all = consts.tile([128, NT], F32)

        o_view = out_o.rearrange("(t p f) c -> t p (f c)", t=8, p=128, f=8)
        s_view = out_s.rearrange("(t p f) c -> t p (f c)", t=8, p=128, f=8)

        # ---- emission helpers -------------------------------------------
        def emit_x_dma(c):
            xs = []
            for kk in range(2):
                x_t = xpool.tile(
                    [128, 128 * NB], F32R, tag=f"x{kk}", name=f"x{kk}"
                )
                ks = slice(128 * kk, 128 * (kk + 1))
                for q in range(2):
                    hs = slice(
                        128 * NB * c + 1024 * q, 128 * NB * c + 1024 * (q + 1)
                    )
                    nc.sync.dma_start(out=x_t[:, 1024 * q : 1024 * (q + 1)],
                                      in_=xTh[ks, hs])
                xs.append(x_t)
            return xs

        def emit_btile(s_chunk, xs, c, k):
            kb = slice(128 * k, 128 * (k + 1))
            z = zpool.tile([128, NODES], F32, tag="z")
            for kk in range(2):
                for nh in range(2):
                    ns = slice(512 * nh, 512 * (nh + 1))
                    nc.tensor.matmul(
                        z[:, ns], xs[kk][:, kb], wh[kk][:, ns],
                        start=(kk == 0), stop=(kk == 1),
                    )
            if bias_path:
                nc.vector.tensor_tensor(
                    out=s_chunk[:, k, :], in0=z, in1=bth, op=A.add
                )
            elif k % 8 == 3:
                # DVE takes 2 of 16 evictions; ACT the rest
                nc.vector.tensor_copy(out=s_chunk[:, k, :], in_=z)
            else:
                nc.scalar.copy(out=s_chunk[:, k, :], in_=z)

        class Descent:
            """Per-chunk descent state; stages emitted interleaved."""

            def __init__(self, c, s_chunk):
                self.c = c
                self.s = s_chunk
                self.sflat = s_chunk.rearrange("p b n -> p (b n)")

            def emit_dp(self):
                s, c = self.s, self.c
                # DP over levels 0..4 -> col5 (visited level-5 column).
                u = dpool.tile([128, NB, 16], F32, tag="u4", name="u4")
                nc.vector.tensor_scalar(
                    out=u, in0=s[:, :, 16:32], scalar1=0.0, scalar2=None,
                    op0=A.is_lt,
                )
                r = u
                for j in range(3, -1, -1):
                    n = 1 << j
                    uj = dpool.tile([128, NB, n], F32, tag=f"u{j}", name=f"u{j}")
                    nc.vector.tensor_scalar(
                        out=uj, in0=s[:, :, n : 2 * n], scalar1=0.0,
                        scalar2=None, op0=A.is_lt,
                    )
                    rp = r.rearrange("p b (n two) -> p b n two", two=2)
                    d = dpool.tile([128, NB, n], F32, tag=f"d{j}", name=f"d{j}")
                    nc.vector.scalar_tensor_tensor(
                        out=d, in0=rp[:, :, :, 1], scalar=float(1 << (4 - j)),
                        in1=rp[:, :, :, 0], op0=A.add, op1=A.subtract,
                    )
                    nc.vector.tensor_tensor(out=d, in0=uj, in1=d, op=A.mult)
                    rn = dpool.tile([128, NB, n], F32, tag=f"r{j}", name=f"r{j}")
                    nc.vector.tensor_tensor(
                        out=rn, in0=rp[:, :, :, 0], in1=d, op=A.add
                    )
                    r = rn
                # col5 = 32 + c5 (int16)
                col = dpool.tile([128, NB], I16, tag="col5", name="col5")
                nc.vector.tensor_scalar(
                    out=col, in0=r[:, :, 0], scalar1=32.0, scalar2=None,
                    op0=A.add,
                )
                self.col = col
                # check-gather indices for levels 0..4: (col5 >> (5-j)) + b*1024
                # check-gather indices: visited columns at levels 0..4,
                # peeled from c5 = col5 - 32 with exact integer compare ops.
                pidx = dpool.tile([128, NB, 5], I16, tag="pidx", name="pidx")
                nc.vector.tensor_scalar(
                    out=pidx[:, :, 0], in0=b1024, scalar1=1, scalar2=None,
                    op0=A.add,
                )
                v = dpool.tile([128, NB], I16, tag="v4", name="v4")
                nc.vector.tensor_scalar(
                    out=v, in0=col, scalar1=-32, scalar2=None, op0=A.add
                )
                colp = None
                for j in range(4):
                    kk = 16 >> j
                    bj = dpool.tile([128, NB], I16, tag=f"pb{j}", name=f"pb{j}")
                    nc.vector.tensor_scalar(
                        out=bj, in0=v, scalar1=kk, scalar2=None, op0=A.is_ge
                    )
                    v2 = dpool.tile([128, NB], I16, tag=f"pv{j}", name=f"pv{j}")
                    nc.vector.scalar_tensor_tensor(
                        out=v2, in0=bj, scalar=-kk, in1=v, op0=A.mult, op1=A.add
                    )
                    v = v2
                    cp = dpool.tile([128, NB], I16, tag=f"pc{j}", name=f"pc{j}")
                    if j == 0:
                        nc.vector.tensor_scalar(
                            out=cp, in0=bj, scalar1=2, scalar2=None, op0=A.add
                        )
                    else:
                        nc.vector.scalar_tensor_tensor(
                            out=cp, in0=colp, scalar=2, in1=bj,
                            op0=A.mult, op1=A.add,
                        )
                    colp = cp
                    nc.vector.tensor_tensor(
                        out=pidx[:, :, j + 1], in0=colp, in1=b1024, op=A.add
                    )
                self.pidx = pidx

            def emit_round(self, ell):
                # level ell in 5..9: gather z at current col, take sign
                idx = dpool.tile([128, NB], I16, tag=f"idx{ell}", name=f"idx{ell}")
                nc.vector.tensor_tensor(
                    out=idx, in0=self.col, in1=b1024, op=A.add
                )
                wide = gpool.tile([128, 256], F32, tag="w", name=f"w{ell}")
                nc.gpsimd.ap_gather(
                    out_ap=wide, in_ap=self.sflat, idxs_ap=idx,
                    channels=128, num_elems=NB * NODES, d=1, num_idxs=256,
                )
                mskd = gpool.tile([128, 256], F32, tag="m", name=f"m{ell}")
                nc.vector.tensor_tensor(out=mskd, in0=wide, in1=mask16, op=A.mult)
                g = dpool.tile([128, NB], F32, tag=f"g{ell}", name=f"g{ell}")
                nc.vector.tensor_reduce(
                    out=g, in_=mskd.rearrange("p (b i) -> p b i", i=16),
                    axis=mybir.AxisListType.X, op=A.add,
                )
                ga = dpool.tile([128, NB], F32, tag=f"ga{ell}", name=f"ga{ell}")
                nc.vector.tensor_reduce(
                    out=ga, in_=g.rearrange("p (b one) -> p b one", one=1),
                    axis=mybir.AxisListType.X, op=A.min,
                    apply_absolute_value=True,
                )
                if ell == 5:
                    self.mn = ga
                else:
                    mn2 = dpool.tile([128, NB], F32, tag=f"mn{ell}", name=f"mn{ell}")
                    nc.vector.tensor_tensor(out=mn2, in0=self.mn, in1=ga, op=A.min)
                    self.mn = mn2
                b = dpool.tile([128, NB], I16, tag=f"b{ell}", name=f"b{ell}")
                nc.vector.tensor_scalar(
                    out=b, in0=g, scalar1=0.0, scalar2=None, op0=A.is_lt
                )
                col2 = dpool.tile([128, NB], I16, tag=f"c{ell}", name=f"c{ell}")
                nc.vector.scalar_tensor_tensor(
                    out=col2, in0=self.col, scalar=2, in1=b, op0=A.mult, op1=A.add
                )
                self.col = col2

            def emit_check(self):
                c = self.c
                widec = gpool.tile([128, 1280], F32, tag="wc", name="wc")
                nc.gpsimd.ap_gather(
                    out_ap=widec, in_ap=self.sflat,
                    idxs_ap=self.pidx.rearrange("p b j -> p (b j)"),
                    channels=128, num_elems=NB * NODES, d=1, num_idxs=1280,
                )
                nc.vector.tensor_tensor(out=widec, in0=widec, in1=bigm, op=A.add)
                mnc = dpool.tile([128, NB], F32, tag="mnc", name="mnc")
                nc.vector.tensor_reduce(
                    out=mnc, in_=widec.rearrange("p (b i) -> p b i", i=80),
                    axis=mybir.AxisListType.X, op=A.min,
                    apply_absolute_value=True,
                )
                nc.vector.tensor_tensor(
                    out=mn_all[:, NB * c : NB * (c + 1)], in0=self.mn, in1=mnc,
                    op=A.min,
                )

            def emit_tables(self):
                c = self.c
                leaf = dpool.tile([128, NB], I16, tag="leaf", name="leaf")
                nc.vector.tensor_scalar(
                    out=leaf, in0=self.col, scalar1=-1024, scalar2=None,
                    op0=A.add,
                )
                rs = slice(256 * c, 256 * (c + 1))
                for tbl, rbuf in ((t_out, r_out), (t_std, r_std)):
                    nc.gpsimd.ap_gather(
                        out_ap=rbuf[:, rs], in_ap=tbl, idxs_ap=leaf,
                        channels=128, num_elems=NODES, d=1, num_idxs=256,
                    )

            def emit_out(self):
                c = self.c
                for h in range(2):
                    bb = 2 * c + h
                    bs = slice(256 * c + 128 * h, 256 * c + 128 * (h + 1))
                    for rbuf, dview in ((r_out, o_view), (r_std, s_view)):
                        pt = tpool.tile([128, 128], F32, tag="t", name="pt")
                        nc.tensor.transpose(pt, rbuf[:, bs], ident)
                        rt = opool.tile([128, 128], F32, tag="rt", name="rt")
                        nc.vector.tensor_copy(out=rt, in_=pt)
                        nc.sync.dma_start(out=dview[bb], in_=rt)

        # ---- main loop: chunk c matmuls interleaved with chunk c-1 descent
        prev = None
        xs = emit_x_dma(0)
        xs_next = None
        for c in range(NCH):
            s_chunk = spool.tile([128, NB, NODES], F32, tag="s", name="s")
            for k in range(NB):
                emit_btile(s_chunk, xs, c, k)
                if c == 0 and k == 0:
                    load_late_consts()
                if k == 5 and c + 1 < NCH:
                    xs_next = emit_x_dma(c + 1)
                if prev is not None:
                    if k == 0:
                        prev.emit_dp()
                    elif k in (2, 4, 6, 8, 10):
                        prev.emit_round(4 + k // 2)
                    elif k == 11:
                        prev.emit_check()
                    elif k == 13:
                        prev.emit_tables()
                    elif k == 15:
                        prev.emit_out()
            prev = Descent(c, s_chunk)
            xs = xs_next

        # tail: descent of the last chunk
        prev.emit_dp()
        for ell in range(5, 10):
            prev.emit_round(ell)
        prev.emit_check()
        prev.emit_tables()
        prev.emit_out()
        nc.sync.dma_start(out=out_m[:, :], in_=mn_all)

    nc.compile()
    return nc


_CACHE = {}


def _get_nc(bias_path: bool):
    if bias_path not in _CACHE:
        nc = bacc.Bacc("TRN2", target_bir_lowering=False, debug=False)
        _CACHE[bias_path] = _build(nc, bias_path)
    return _CACHE[bias_path]


# Within each 128-row block, device partition p holds sample row PERM[p]
# (aligns ap_gather's per-core wrapped indices with natural output order).
PERM = np.array([8 * (p % 16) + p // 16 for p in range(128)], dtype=np.int64)


def _e8m11(x):
    """Round fp32 to the HW fp32r format (8-bit exp, 11-bit mantissa, RNE)."""
    u = np.ascontiguousarray(x, np.float32).view(np.uint32)
    low = u & np.uint32(0xFFF)
    base = u & np.uint32(0xFFFFF000)
    add = (low > 0x800) | ((low == 0x800) & ((u >> 12) & 1).astype(bool))
    return (base + np.where(add, np.uint32(0x1000), np.uint32(0))).view(np.float32)


def _shard_xT(x_shard):
    """[8192, 256] sample rows -> permuted, transposed [256, 8192] device input."""
    xp = x_shard.reshape(NT, 128, IN_DIM)[:, PERM, :].reshape(B_LOC, IN_DIM)
    return np.ascontiguousarray(xp.T)


def _prepare(x, W_pred, b_pred, W_or, action_stds):
    x = np.ascontiguousarray(x, dtype=np.float32)
    W_pred = np.asarray(W_pred, dtype=np.float32)
    b_pred = np.asarray(b_pred, dtype=np.float32)
    W_or = np.asarray(W_or, dtype=np.float32)
    action_stds = np.asarray(action_stds, dtype=np.float32)

    n_int = 2**HEIGHT - 1
    Wp = np.zeros((IN_DIM, NODES), np.float32)
    Wp[:, 1 : n_int + 1] = W_pred.T          # col j+1 = node j; col 0 = pad
    Wph = _e8m11(Wp)
    m = W_or.max(axis=0, keepdims=True)
    e = np.exp(W_or - m)
    t_out16 = (e / e.sum(axis=0, keepdims=True)).astype(np.float32)  # [16,1024]
    t_std16 = np.clip(action_stds, -20.0, 2.0).astype(np.float32)
    t_out = np.tile(t_out16, (8, 1))
    t_std = np.tile(t_std16, (8, 1))

    pcol = np.arange(128)[:, None]
    mask16 = (np.arange(256)[None, :] % 16 == pcol % 16).astype(np.float32)
    bigm = np.where(np.arange(1280)[None, :] % 16 == pcol % 16, 0.0, 1000.0
                    ).astype(np.float32)
    b1024 = np.tile((np.arange(NB) * 1024).astype(np.int16), (128, 1))

    bth = np.zeros((128, NODES), np.float32)
    bth[:, 1 : n_int + 1] = b_pred[None, :]
    bias_path = bool(np.any(b_pred != 0.0))
    return (x, W_pred, b_pred, t_out16, t_std16, Wph, t_out, t_std, mask16,
            bigm, b1024, bth, bias_path)


def _host_fixup(out, std, flags, x, W_pred, b_pred, t_out16, t_std16):
    rows = np.nonzero(flags)[0]
    if len(rows) == 0:
        return 0
    z = x[rows].astype(np.float64) @ W_pred.T.astype(np.float64) \
        + b_pred.astype(np.float64)
    col = np.ones(len(rows), dtype=np.int64)
    for _ in range(HEIGHT):
        zc = z[np.arange(len(rows)), col - 1]
        col = 2 * col + (zc < 0)
    leaf = col - 1024
    out[rows] = t_out16[:, leaf].T
    std[rows] = t_std16[:, leaf].T
    return len(rows)


def kernel(x, W_pred, b_pred, W_or, action_stds, _want_trace=False):
    (x, W_pred, b_pred, t_out16, t_std16, Wph, t_out, t_std, mask16, bigm,
     b1024, bth, bias_path) = _prepare(x, W_pred, b_pred, W_or, action_stds)
    nc = _get_nc(bias_path)

    in_maps = []
    for c in range(N_CORES):
        shard = x[c * B_LOC : (c + 1) * B_LOC]
        im = {
            "xTh": _e8m11(_shard_xT(shard)),
            "Wph": Wph,
            "Tout": t_out,
            "Tstd": t_std,
            "Ident": np.eye(128, dtype=np.float32),
            "Mask16": mask16,
            "BigM": bigm,
            "B1024": b1024,
        }
        if bias_path:
            im["BTH"] = bth
        in_maps.append(im)

    res = run_bass_kernel_spmd(
        nc, in_maps, core_ids=list(range(N_CORES)), trace=_want_trace
    )
    out = np.concatenate([res.results[c]["out_o"] for c in range(N_CORES)], axis=0)
    std = np.concatenate([res.results[c]["out_s"] for c in range(N_CORES)], axis=0)

    # min-path-|z| -> per-sample flags (undo the PERM row permutation)
    flags = np.empty(BATCH, dtype=bool)
    for c in range(N_CORES):
        mn = np.asarray(res.results[c]["out_m"])          # [128, NT]
        fl = mn < TAU                                      # [p, t]
        fl_rows = np.empty((NT, 128), dtype=bool)
        fl_rows[:, PERM] = fl.T                            # sample t*128+PERM[p]
        flags[c * B_LOC : (c + 1) * B_LOC] = fl_rows.reshape(-1)
    nfix = _host_fixup(out, std, flags, x, W_pred, b_pred, t_out16, t_std16)
    kernel.last_nfix = nfix
    if _want_trace:
        kernel.last_results = res
    return out, std


# revision 24
# speedup vs baseline: 1.1963x; 1.1963x over previous
"""Trainium2 Bass kernel for nn_DGT_6485400616966 (soft decision tree forward).

Math (forward pass only):
  pred_z = x @ W_pred.T + b_pred                      [B, 1023]
  The straight-through/one-hot structure collapses: the output depends only on
  the argmax leaf of the tree AND layer, which equals a 10-level tree descent
  following sign(pred_z) at visited nodes (left if z >= 0).
  out = softmax(W_or[:, leaf]) ; std = clip(action_stds[:, leaf], -20, 2)

Device algorithm per core (8192 samples, data-parallel over 8 cores):
  1. PE: z = x @ W_pred.T in ONE fp32r (e8m11) pass. Error |dz| <= ~1e-3;
     rows whose descent passes within TAU=2e-3 of a sign flip are flagged via
     a min-|z|-along-path output and recomputed exactly on the host (~1.7% of
     rows, zero observed misses with 2x margin; see work/check_numerics.py).
  2. Eviction PSUM->SBUF per btile: z kept as fp32 (ap_gather needs 4-byte
     elements for d=1), split ACT (scalar copy) / DVE (tensor copy).
  3. Tree descent per chunk of 16 btiles:
     - DVE DP over levels 0..4 (31 nodes) -> visited level-5 column col5.
     - 5 sequential rounds for levels 5..9: GPSIMD ap_gather of z[col] with
       per-core-wrapped indices (out[p, 16t+p%16] = z[p, idx[p,t]]), DVE
       mask-multiply + reduce extraction, sign -> next column.
     - check gather of the level 0..4 path columns; min |z| over the full
       path -> out_m (host flag).
  4. GPSIMD ap_gather table lookups T[class, leaf] with classes replicated on
     partitions (host pre-permutes rows by pi(p)=8*(p%16)+p//16 so outputs
     land in natural order), PE transpose, contiguous DMA out.
"""

import sys

for _p in ("/opt/trn_rl_repo",):
    if _p not in sys.path:
        sys.path.insert(0, _p)

from contextlib import ExitStack

import numpy as np

import concourse.bacc as bacc
import concourse.bass as bass
import concourse.tile as tile
from concourse import mybir
from concourse.bass_utils import run_bass_kernel_spmd

HEIGHT = 10
IN_DIM = 256
OUT_DIM = 16
BATCH = 65536
N_CORES = 8
B_LOC = BATCH // N_CORES          # 8192 samples per core
NT = B_LOC // 128                 # 64 batch tiles of 128 samples
NB = 16                           # btiles per descent chunk
NCH = NT // NB                    # 4 chunks
NODES = 1024                      # col j+1 = tree node j; col 0 = zero pad
TAU = 2e-3                        # host-fixup flag threshold on min path |z|
F32 = mybir.dt.float32
F32R = mybir.dt.float32r
I16 = mybir.dt.int16
A = mybir.AluOpType


def _build(nc, bias_path: bool):
    xTh = nc.dram_tensor("xTh", [IN_DIM, B_LOC], F32R, kind="ExternalInput")
    Wph = nc.dram_tensor("Wph", [IN_DIM, NODES], F32R, kind="ExternalInput")
    Tout = nc.dram_tensor("Tout", [128, NODES], F32, kind="ExternalInput")
    Tstd = nc.dram_tensor("Tstd", [128, NODES], F32, kind="ExternalInput")
    Ident = nc.dram_tensor("Ident", [128, 128], F32, kind="ExternalInput")
    Mask16 = nc.dram_tensor("Mask16", [128, 256], F32, kind="ExternalInput")
    BigM = nc.dram_tensor("BigM", [128, 1280], F32, kind="ExternalInput")
    B1024 = nc.dram_tensor("B1024", [128, NB], I16, kind="ExternalInput")
    BTH = None
    if bias_path:
        BTH = nc.dram_tensor("BTH", [128, NODES], F32, kind="ExternalInput")
    out_o = nc.dram_tensor("out_o", [B_LOC, OUT_DIM], F32, kind="ExternalOutput")
    out_s = nc.dram_tensor("out_s", [B_LOC, OUT_DIM], F32, kind="ExternalOutput")
    out_m = nc.dram_tensor("out_m", [128, NT], F32, kind="ExternalOutput")

    with tile.TileContext(nc) as tc, ExitStack() as ctx:
        consts = ctx.enter_context(tc.tile_pool(name="consts", bufs=1))
        xpool = ctx.enter_context(tc.tile_pool(name="xpool", bufs=2))
        spool = ctx.enter_context(tc.tile_pool(name="spool", bufs=2))
        dpool = ctx.enter_context(tc.tile_pool(name="dpool", bufs=1))
        gpool = ctx.enter_context(tc.tile_pool(name="gpool", bufs=1))
        opool = ctx.enter_context(tc.tile_pool(name="opool", bufs=2))
        zpool = ctx.enter_context(
            tc.tile_pool(name="zpool", bufs=3, space=bass.MemorySpace.PSUM)
        )
        tpool = ctx.enter_context(
            tc.tile_pool(name="tpool", bufs=2, space=bass.MemorySpace.PSUM)
        )

        wh = [
            consts.tile([128, NODES], F32R, tag=f"wh{k}", name=f"wh{k}")
            for k in range(2)
        ]
        for k in range(2):
            nc.sync.dma_start(out=wh[k], in_=Wph[128 * k : 128 * (k + 1), :])

        t_out = consts.tile([128, NODES], F32)
        t_std = consts.tile([128, NODES], F32)
        ident = consts.tile([128, 128], F32)
        mask16 = consts.tile([128, 256], F32)
        bigm = consts.tile([128, 1280], F32)
        b1024 = consts.tile([128, NB], I16)
        bth = consts.tile([128, NODES], F32) if bias_path else None

        nc.sync.dma_start(out=ident, in_=Ident[:, :])

        def load_late_consts():
            nc.sync.dma_start(out=t_out, in_=Tout[:, :])
            nc.sync.dma_start(out=t_std, in_=Tstd[:, :])
            nc.sync.dma_start(out=mask16, in_=Mask16[:, :])
            nc.sync.dma_start(out=bigm, in_=BigM[:, :])
            nc.sync.dma_start(out=b1024, in_=B1024[:, :])
            if bias_path:
                nc.sync.dma_start(out=bth, in_=BTH[:, :])

        r_out = consts.tile([128, NODES], F32)
        r_std = consts.tile([128, NODES], F32)
        mn_all = consts.tile([128, NT], F32)

        o_view = out_o.rearrange("(t p f) c -> t p (f c)", t=8, p=128, f=8)
        s_view = out_s.rearrange("(t p f) c -> t p (f c)", t=8, p=128, f=8)

        # ---- emission helpers -------------------------------------------
        def emit_x_dma(c):
            xs = []
            for kk in range(2):
                x_t = xpool.tile(
                    [128, 128 * NB], F32R, tag=f"x{kk}", name=f"x{kk}"
                )
                ks = slice(128 * kk, 128 * (kk + 1))
                for q in range(2):
                    hs = slice(
                        128 * NB * c + 1024 * q, 128 * NB * c + 1024 * (q + 1)
                    )
                    nc.sync.dma_start(out=x_t[:, 1024 * q : 1024 * (q + 1)],
                                      in_=xTh[ks, hs])
                xs.append(x_t)
            return xs

        def emit_btile(s_chunk, xs, c, k):
            kb = slice(128 * k, 128 * (k + 1))
            z = zpool.tile([128, NODES], F32, tag="z")
            for kk in range(2):
                for nh in range(2):
                    ns = slice(512 * nh, 512 * (nh + 1))
                    nc.tensor.matmul(
                        z[:, ns], xs[kk][:, kb], wh[kk][:, ns],
                        start=(kk == 0), stop=(kk == 1),
                    )
            if bias_path:
                nc.vector.tensor_tensor(
                    out=s_chunk[:, k, :], in0=z, in1=bth, op=A.add
                )
            elif k in (3, 5, 7, 9, 11):
                # DVE evictions interleave 1:1 with descent gather rounds so
                # the DVE FIFO never head-blocks on a GPSIMD gather.
                nc.vector.tensor_copy(out=s_chunk[:, k, :], in_=z)
            else:
                nc.scalar.copy(out=s_chunk[:, k, :], in_=z)

        class Descent:
            """Per-chunk descent state; stages emitted interleaved."""

            def __init__(self, c, s_chunk):
                self.c = c
                self.s = s_chunk
                self.sflat = s_chunk.rearrange("p b n -> p (b n)")

            def emit_dp(self):
                s, c = self.s, self.c
                # DP over levels 0..4 -> col5 (visited level-5 column).
                u = dpool.tile([128, NB, 16], F32, tag="u4", name="u4")
                nc.vector.tensor_scalar(
                    out=u, in0=s[:, :, 16:32], scalar1=0.0, scalar2=None,
                    op0=A.is_lt,
                )
                r = u
                for j in range(3, -1, -1):
                    n = 1 << j
                    uj = dpool.tile([128, NB, n], F32, tag=f"u{j}", name=f"u{j}")
                    nc.vector.tensor_scalar(
                        out=uj, in0=s[:, :, n : 2 * n], scalar1=0.0,
                        scalar2=None, op0=A.is_lt,
                    )
                    rp = r.rearrange("p b (n two) -> p b n two", two=2)
                    d = dpool.tile([128, NB, n], F32, tag=f"d{j}", name=f"d{j}")
                    nc.vector.scalar_tensor_tensor(
                        out=d, in0=rp[:, :, :, 1], scalar=float(1 << (4 - j)),
                        in1=rp[:, :, :, 0], op0=A.add, op1=A.subtract,
                    )
                    nc.vector.tensor_tensor(out=d, in0=uj, in1=d, op=A.mult)
                    rn = dpool.tile([128, NB, n], F32, tag=f"r{j}", name=f"r{j}")
                    nc.vector.tensor_tensor(
                        out=rn, in0=rp[:, :, :, 0], in1=d, op=A.add
                    )
                    r = rn
                # col5 = 32 + c5 (int16)
                col = dpool.tile([128, NB], I16, tag="col5", name="col5")
                nc.vector.tensor_scalar(
                    out=col, in0=r[:, :, 0], scalar1=32.0, scalar2=None,
                    op0=A.add,
                )
                self.col = col
                # check-gather indices for levels 0..4: (col5 >> (5-j)) + b*1024
                # check-gather indices: visited columns at levels 0..4,
                # peeled from c5 = col5 - 32 with exact integer compare ops.
                pidx = dpool.tile([128, NB, 5], I16, tag="pidx", name="pidx")
                nc.vector.tensor_scalar(
                    out=pidx[:, :, 0], in0=b1024, scalar1=1, scalar2=None,
                    op0=A.add,
                )
                v = dpool.tile([128, NB], I16, tag="v4", name="v4")
                nc.vector.tensor_scalar(
                    out=v, in0=col, scalar1=-32, scalar2=None, op0=A.add
                )
                colp = None
                for j in range(4):
                    kk = 16 >> j
                    bj = dpool.tile([128, NB], I16, tag=f"pb{j}", name=f"pb{j}")
                    nc.vector.tensor_scalar(
                        out=bj, in0=v, scalar1=kk, scalar2=None, op0=A.is_ge
                    )
                    v2 = dpool.tile([128, NB], I16, tag=f"pv{j}", name=f"pv{j}")
                    nc.vector.scalar_tensor_tensor(
                        out=v2, in0=bj, scalar=-kk, in1=v, op0=A.mult, op1=A.add
                    )
                    v = v2
                    cp = dpool.tile([128, NB], I16, tag=f"pc{j}", name=f"pc{j}")
                    if j == 0:
                        nc.vector.tensor_scalar(
                            out=cp, in0=bj, scalar1=2, scalar2=None, op0=A.add
                        )
                    else:
                        nc.vector.scalar_tensor_tensor(
                            out=cp, in0=colp, scalar=2, in1=bj,
                            op0=A.mult, op1=A.add,
                        )
                    colp = cp
                    nc.vector.tensor_tensor(
                        out=pidx[:, :, j + 1], in0=colp, in1=b1024, op=A.add
                    )
                self.pidx = pidx

            def emit_round(self, ell):
                # level ell in 5..9: gather z at current col, take sign
                idx = dpool.tile([128, NB], I16, tag=f"idx{ell}", name=f"idx{ell}")
                nc.vector.tensor_tensor(
                    out=idx, in0=self.col, in1=b1024, op=A.add
                )
                wide = gpool.tile([128, 256], F32, tag="w", name=f"w{ell}")
                nc.gpsimd.ap_gather(
                    out_ap=wide, in_ap=self.sflat, idxs_ap=idx,
                    channels=128, num_elems=NB * NODES, d=1, num_idxs=256,
                )
                mskd = gpool.tile([128, 256], F32, tag="m", name=f"m{ell}")
                nc.vector.tensor_tensor(out=mskd, in0=wide, in1=mask16, op=A.mult)
                g = dpool.tile([128, NB], F32, tag=f"g{ell}", name=f"g{ell}")
                nc.vector.tensor_reduce(
                    out=g, in_=mskd.rearrange("p (b i) -> p b i", i=16),
                    axis=mybir.AxisListType.X, op=A.add,
                )
                ga = dpool.tile([128, NB], F32, tag=f"ga{ell}", name=f"ga{ell}")
                nc.vector.tensor_reduce(
                    out=ga, in_=g.rearrange("p (b one) -> p b one", one=1),
                    axis=mybir.AxisListType.X, op=A.min,
                    apply_absolute_value=True,
                )
                if ell == 5:
                    self.mn = ga
                else:
                    mn2 = dpool.tile([128, NB], F32, tag=f"mn{ell}", name=f"mn{ell}")
                    nc.vector.tensor_tensor(out=mn2, in0=self.mn, in1=ga, op=A.min)
                    self.mn = mn2
                b = dpool.tile([128, NB], I16, tag=f"b{ell}", name=f"b{ell}")
                nc.vector.tensor_scalar(
                    out=b, in0=g, scalar1=0.0, scalar2=None, op0=A.is_lt
                )
                col2 = dpool.tile([128, NB], I16, tag=f"c{ell}", name=f"c{ell}")
                nc.vector.scalar_tensor_tensor(
                    out=col2, in0=self.col, scalar=2, in1=b, op0=A.mult, op1=A.add
                )
                self.col = col2

            def emit_check(self):
                c = self.c
                widec = gpool.tile([128, 1280], F32, tag="wc", name="wc")
                nc.gpsimd.ap_gather(
                    out_ap=widec, in_ap=self.sflat,
                    idxs_ap=self.pidx.rearrange("p b j -> p (b j)"),
                    channels=128, num_elems=NB * NODES, d=1, num_idxs=1280,
                )
                nc.vector.tensor_tensor(out=widec, in0=widec, in1=bigm, op=A.add)
                mnc = dpool.tile([128, NB], F32, tag="mnc", name="mnc")
                nc.vector.tensor_reduce(
                    out=mnc, in_=widec.rearrange("p (b i) -> p b i", i=80),
                    axis=mybir.AxisListType.X, op=A.min,
                    apply_absolute_value=True,
                )
                nc.vector.tensor_tensor(
                    out=mn_all[:, NB * c : NB * (c + 1)], in0=self.mn, in1=mnc,
                    op=A.min,
                )

            def emit_tables(self):
                c = self.c
                leaf = dpool.tile([128, NB], I16, tag="leaf", name="leaf")
                nc.vector.tensor_scalar(
                    out=leaf, in0=self.col, scalar1=-1024, scalar2=None,
                    op0=A.add,
                )
                rs = slice(256 * c, 256 * (c + 1))
                for tbl, rbuf in ((t_out, r_out), (t_std, r_std)):
                    nc.gpsimd.ap_gather(
                        out_ap=rbuf[:, rs], in_ap=tbl, idxs_ap=leaf,
                        channels=128, num_elems=NODES, d=1, num_idxs=256,
                    )

            def emit_out(self):
                c = self.c
                for h in range(2):
                    bb = 2 * c + h
                    bs = slice(256 * c + 128 * h, 256 * c + 128 * (h + 1))
                    for rbuf, dview in ((r_out, o_view), (r_std, s_view)):
                        pt = tpool.tile([128, 128], F32, tag="t", name="pt")
                        nc.tensor.transpose(pt, rbuf[:, bs], ident)
                        rt = opool.tile([128, 128], F32, tag="rt", name="rt",
                                        bufs=4)
                        nc.vector.tensor_copy(out=rt, in_=pt)
                        nc.sync.dma_start(out=dview[bb], in_=rt)

        # warmup: preload the ap_gather Q7 library (~6us IRAM load) before
        # the first real gather needs it.
        warm_idx = consts.tile([128, 1], I16)
        nc.gpsimd.memset(warm_idx, 0)
        warm_out = consts.tile([128, 16], F32)
        nc.gpsimd.ap_gather(
            out_ap=warm_out, in_ap=ident, idxs_ap=warm_idx,
            channels=128, num_elems=128, d=1, num_idxs=16,
        )

        # ---- main loop: chunk c matmuls interleaved with chunk c-1 descent
        done = []
        prev = None
        xs = emit_x_dma(0)
        xs_next = None
        for c in range(NCH):
            s_chunk = spool.tile([128, NB, NODES], F32, tag="s", name="s")
            for k in range(NB):
                emit_btile(s_chunk, xs, c, k)
                if c == 0 and k == 0:
                    load_late_consts()
                if k == 5 and c + 1 < NCH:
                    xs_next = emit_x_dma(c + 1)
                if prev is not None:
                    if k == 0:
                        prev.emit_dp()
                    elif k in (2, 4, 6, 8, 10):
                        prev.emit_round(4 + k // 2)
                    elif k == 11:
                        prev.emit_check()
                    elif k == 13:
                        prev.emit_tables()
            if prev is not None:
                done.append(prev)
            prev = Descent(c, s_chunk)
            xs = xs_next

        # tail: descent of the last chunk, then all output transposes (kept
        # off the PE queue until every matmul has issued).
        prev.emit_dp()
        for ell in range(5, 10):
            prev.emit_round(ell)
        prev.emit_check()
        prev.emit_tables()
        done.append(prev)
        for d_ in done:
            d_.emit_out()
        nc.sync.dma_start(out=out_m[:, :], in_=mn_all)

    nc.compile()
    return nc


_CACHE = {}


def _get_nc(bias_path: bool):
    if bias_path not in _CACHE:
        nc = bacc.Bacc("TRN2", target_bir_lowering=False, debug=False)
        _CACHE[bias_path] = _build(nc, bias_path)
    return _CACHE[bias_path]


# Within each 128-row block, device partition p holds sample row PERM[p]
# (aligns ap_gather's per-core wrapped indices with natural output order).
PERM = np.array([8 * (p % 16) + p // 16 for p in range(128)], dtype=np.int64)


def _e8m11(x):
    """Round fp32 to the HW fp32r format (8-bit exp, 11-bit mantissa, RNE)."""
    u = np.ascontiguousarray(x, np.float32).view(np.uint32)
    low = u & np.uint32(0xFFF)
    base = u & np.uint32(0xFFFFF000)
    add = (low > 0x800) | ((low == 0x800) & ((u >> 12) & 1).astype(bool))
    return (base + np.where(add, np.uint32(0x1000), np.uint32(0))).view(np.float32)


def _shard_xT(x_shard):
    """[8192, 256] sample rows -> permuted, transposed [256, 8192] device input."""
    xp = x_shard.reshape(NT, 128, IN_DIM)[:, PERM, :].reshape(B_LOC, IN_DIM)
    return np.ascontiguousarray(xp.T)


def _prepare(x, W_pred, b_pred, W_or, action_stds):
    x = np.ascontiguousarray(x, dtype=np.float32)
    W_pred = np.asarray(W_pred, dtype=np.float32)
    b_pred = np.asarray(b_pred, dtype=np.float32)
    W_or = np.asarray(W_or, dtype=np.float32)
    action_stds = np.asarray(action_stds, dtype=np.float32)

    n_int = 2**HEIGHT - 1
    Wp = np.zeros((IN_DIM, NODES), np.float32)
    Wp[:, 1 : n_int + 1] = W_pred.T          # col j+1 = node j; col 0 = pad
    Wph = _e8m11(Wp)
    m = W_or.max(axis=0, keepdims=True)
    e = np.exp(W_or - m)
    t_out16 = (e / e.sum(axis=0, keepdims=True)).astype(np.float32)  # [16,1024]
    t_std16 = np.clip(action_stds, -20.0, 2.0).astype(np.float32)
    t_out = np.tile(t_out16, (8, 1))
    t_std = np.tile(t_std16, (8, 1))

    pcol = np.arange(128)[:, None]
    mask16 = (np.arange(256)[None, :] % 16 == pcol % 16).astype(np.float32)
    bigm = np.where(np.arange(1280)[None, :] % 16 == pcol % 16, 0.0, 1000.0
                    ).astype(np.float32)
    b1024 = np.tile((np.arange(NB) * 1024).astype(np.int16), (128, 1))

    bth = np.zeros((128, NODES), np.float32)
    bth[:, 1 : n_int + 1] = b_pred[None, :]
    bias_path = bool(np.any(b_pred != 0.0))
    return (x, W_pred, b_pred, t_out16, t_std16, Wph, t_out, t_std, mask16,
            bigm, b1024, bth, bias_path)


def _host_fixup(out, std, flags, x, W_pred, b_pred, t_out16, t_std16):
    rows = np.nonzero(flags)[0]
    if len(rows) == 0:
        return 0
    z = x[rows].astype(np.float64) @ W_pred.T.astype(np.float64) \
        + b_pred.astype(np.float64)
    col = np.ones(len(rows), dtype=np.int64)
    for _ in range(HEIGHT):
        zc = z[np.arange(len(rows)), col - 1]
        col = 2 * col + (zc < 0)
    leaf = col - 1024
    out[rows] = t_out16[:, leaf].T
    std[rows] = t_std16[:, leaf].T
    return len(rows)


def kernel(x, W_pred, b_pred, W_or, action_stds, _want_trace=False):
    (x, W_pred, b_pred, t_out16, t_std16, Wph, t_out, t_std, mask16, bigm,
     b1024, bth, bias_path) = _prepare(x, W_pred, b_pred, W_or, action_stds)
    nc = _get_nc(bias_path)

    in_maps = []
    for c in range(N_CORES):
        shard = x[c * B_LOC : (c + 1) * B_LOC]
        im = {
            "xTh": _e8m11(_shard_xT(shard)),
            "Wph": Wph,
            "Tout": t_out,
            "Tstd": t_std,
            "Ident": np.eye(128, dtype=np.float32),
            "Mask16": mask16,
            "BigM": bigm,
            "B1024": b1024,
        }
        if bias_path:
            im["BTH"] = bth
        in_maps.append(im)

    res = run_bass_kernel_spmd(
        nc, in_maps, core_ids=list(range(N_CORES)), trace=_want_trace
    )
    out = np.concatenate([res.results[c]["out_o"] for c in range(N_CORES)], axis=0)
    std = np.concatenate([res.results[c]["out_s"] for c in range(N_CORES)], axis=0)

    # min-path-|z| -> per-sample flags (undo the PERM row permutation)
    flags = np.empty(BATCH, dtype=bool)
    for c in range(N_CORES):
        mn = np.asarray(res.results[c]["out_m"])          # [128, NT]
        fl = mn < TAU                                      # [p, t]
        fl_rows = np.empty((NT, 128), dtype=bool)
        fl_rows[:, PERM] = fl.T                            # sample t*128+PERM[p]
        flags[c * B_LOC : (c + 1) * B_LOC] = fl_rows.reshape(-1)
    nfix = _host_fixup(out, std, flags, x, W_pred, b_pred, t_out16, t_std16)
    kernel.last_nfix = nfix
    if _want_trace:
        kernel.last_results = res
    return out, std


# revision 25
# speedup vs baseline: 2.2006x; 1.8395x over previous
"""Trainium2 Bass kernel for nn_DGT_6485400616966 (soft decision tree forward).

Math (forward pass only):
  pred_z = x @ W_pred.T + b_pred                      [B, 1023]
  The straight-through/one-hot structure collapses: the output depends only on
  the argmax leaf of the tree AND layer, which equals a 10-level tree descent
  following sign(pred_z) at visited nodes (left if z >= 0).
  out = softmax(W_or[:, leaf]) ; std = clip(action_stds[:, leaf], -20, 2)

Device algorithm per core (8192 samples, data-parallel over 8 cores):
  1. PE: z = x @ W_pred.T in three fp32r passes (xh@wh + xh@wl + xl@wh) where
     hi/lo are an exact e8m11 split of the fp32 operands (fp32r on HW is
     e8m11; one pass alone flips ~38 argmax rows, three passes flip none).
     x tiles are the stationary operand; W^T columns (nodes, padded to 1024)
     are the moving operand, N=512 per matmul for full fp32r rate.
  2. Eviction PSUM->SBUF per btile: u = (z < 0) as fp16, contiguous writes
     (strided 2-byte DVE writes cost ~4x). Split DVE tensor_scalar is_lt /
     ACT saturated-sigmoid (Sigmoid(-1e30*z) is exactly {0,1}).
  3. DVE: bottom-up tree collapse r_i = r_e + u_i*(K + r_o - r_e) in fp16 on
     [128, NB, 2^i] chunk tensors (btile-major; all writes contiguous).
  4. GPSIMD ap_gather per chunk: table lookup T[class, leaf] with the 16
     classes replicated on partitions; each 16-partition group shares its
     sample's leaf index (host pre-permutes rows by pi(p)=8*(p%16)+p//16 so
     indices are already wrapped and outputs land in natural order).
  5. PE transpose of the gathered [128, 128] blocks (emitted LAG chunks late
     so the in-order PE queue never stalls) + contiguous DMA out.
"""

import sys

for _p in ("/opt/trn_rl_repo",):
    if _p not in sys.path:
        sys.path.insert(0, _p)

from contextlib import ExitStack

import numpy as np

import concourse.bacc as bacc
import concourse.bass as bass
import concourse.tile as tile
from concourse import mybir
from concourse.bass_utils import run_bass_kernel_spmd

HEIGHT = 10
IN_DIM = 256
OUT_DIM = 16
BATCH = 65536
N_CORES = 8
B_LOC = BATCH // N_CORES          # 8192 samples per core
NT = B_LOC // 128                 # 64 batch tiles of 128 samples
NB = 8                            # btiles per collapse chunk
NCH = NT // NB                    # 8 chunks
NODES = 1024                      # 1023 real + 1 pad
F32 = mybir.dt.float32
F32R = mybir.dt.float32r
BF16 = mybir.dt.bfloat16
FP16 = mybir.dt.float16
I16 = mybir.dt.int16


def _build(nc, use_sign_path: bool):
    """Emit the per-core program. use_sign_path=True assumes b_pred == 0."""
    # hi/lo e8m11 split operands (fp32r is e8m11 on HW; hi+lo == fp32 exactly)
    xTh = nc.dram_tensor("xTh", [IN_DIM, B_LOC], F32R, kind="ExternalInput")
    xTl = nc.dram_tensor("xTl", [IN_DIM, B_LOC], BF16, kind="ExternalInput")
    Wph = nc.dram_tensor("Wph", [IN_DIM, NODES], F32R, kind="ExternalInput")
    Wpl = nc.dram_tensor("Wpl", [IN_DIM, NODES], F32R, kind="ExternalInput")
    Wpb = nc.dram_tensor("Wpb", [IN_DIM, NODES], BF16, kind="ExternalInput")
    Tout = nc.dram_tensor("Tout", [128, NODES], F32, kind="ExternalInput")
    Tstd = nc.dram_tensor("Tstd", [128, NODES], F32, kind="ExternalInput")
    TH = nc.dram_tensor("TH", [128, NODES], F32, kind="ExternalInput")
    Ident = nc.dram_tensor("Ident", [128, 128], F32, kind="ExternalInput")
    out_o = nc.dram_tensor("out_o", [B_LOC, OUT_DIM], F32, kind="ExternalOutput")
    out_s = nc.dram_tensor("out_s", [B_LOC, OUT_DIM], F32, kind="ExternalOutput")

    with tile.TileContext(nc) as tc, ExitStack() as ctx:
        consts = ctx.enter_context(tc.tile_pool(name="consts", bufs=1))
        xpool = ctx.enter_context(tc.tile_pool(name="xpool", bufs=4))
        spool = ctx.enter_context(tc.tile_pool(name="spool", bufs=3))
        rpool = ctx.enter_context(tc.tile_pool(name="rpool", bufs=3))
        dpool = ctx.enter_context(tc.tile_pool(name="dpool", bufs=3))
        zpool = ctx.enter_context(
            tc.tile_pool(name="zpool", bufs=3, space=bass.MemorySpace.PSUM)
        )
        tpool = ctx.enter_context(
            tc.tile_pool(name="tpool", bufs=2, space=bass.MemorySpace.PSUM)
        )

        wh = [
            consts.tile([128, NODES], F32R, tag=f"wh{k}", name=f"wh{k}")
            for k in range(2)
        ]
        wl = [
            consts.tile([128, NODES], F32R, tag=f"wl{k}", name=f"wl{k}")
            for k in range(2)
        ]
        whb = [
            consts.tile([128, NODES], BF16, tag=f"whb{k}", name=f"whb{k}")
            for k in range(2)
        ]
        nc.sync.dma_start(out=wh[0], in_=Wph[0:128, :])
        nc.sync.dma_start(out=whb[0], in_=Wpb[0:128, :])

        def load_late_weights():
            nc.sync.dma_start(out=wl[0], in_=Wpl[0:128, :])
            nc.sync.dma_start(out=wh[1], in_=Wph[128:256, :])
            nc.sync.dma_start(out=wl[1], in_=Wpl[128:256, :])
            nc.sync.dma_start(out=whb[1], in_=Wpb[128:256, :])
        t_out = consts.tile([128, NODES], F32)
        t_std = consts.tile([128, NODES], F32)
        ident = consts.tile([128, 128], F32)
        th = None
        if not use_sign_path:
            th = consts.tile([128, NODES], F32)
            nc.sync.dma_start(out=th, in_=TH[:, :])

        def load_late_consts():
            # tables/identity are first consumed by the descent/output stage;
            # loading them after the first chunk's x keeps the PE start early.
            nc.sync.dma_start(out=t_out, in_=Tout[:, :])
            nc.sync.dma_start(out=t_std, in_=Tstd[:, :])
            nc.sync.dma_start(out=ident, in_=Ident[:, :])

        leaf_all = consts.tile([128, NT], FP16)
        leaf_i16 = consts.tile([128, NT], I16)
        r_out = consts.tile([128, NODES], F32)
        r_std = consts.tile([128, NODES], F32)

        o_view = out_o.rearrange("(t p f) c -> t p (f c)", t=8, p=128, f=8)
        s_view = out_s.rearrange("(t p f) c -> t p (f c)", t=8, p=128, f=8)
        LAG = 3

        def emit_out_chain(cc):
            # transpose chunk cc's gathered [128, 128] table blocks and DMA
            # them out; emitted LAG chunks late so the in-order PE queue
            # never stalls on the descent chain.
            rs_ = slice(128 * cc, 128 * (cc + 1))
            for rbuf, dview in ((r_out, o_view), (r_std, s_view)):
                pt = tpool.tile([128, 128], F32, tag="t", name="pt")
                nc.tensor.transpose(pt, rbuf[:, rs_], ident)
                rt = xpool.tile([128, 128], F32, tag="rt", name="rt", bufs=2)
                nc.scalar.copy(out=rt, in_=pt)
                nc.sync.dma_start(out=dview[cc], in_=rt)

        for c in range(NCH):
            # btile-MAJOR u-bit store: eviction writes [128, 1024] contiguous
            # (strided 2-byte writes cost ~4x on DVE; reads don't).
            s_chunk = spool.tile([128, NB, NODES], FP16, tag="s")
            for k in range(NB):
                t = c * NB + k
                bs = slice(128 * t, 128 * (t + 1))
                if k == 0:
                    # stage x for this chunk: [128, 128*NB] per ktile/half
                    hs = slice(128 * NB * c, 128 * NB * (c + 1))
                    xh = [
                        xpool.tile(
                            [128, 128 * NB], F32R,
                            tag=f"xh{kk}", name=f"xh{kk}", bufs=2,
                        )
                        for kk in range(2)
                    ]
                    xl = [
                        xpool.tile(
                            [128, 128 * NB], BF16,
                            tag=f"xl{kk}", name=f"xl{kk}", bufs=2,
                        )
                        for kk in range(2)
                    ]
                    for kk in range(2):
                        ks = slice(128 * kk, 128 * (kk + 1))
                        nc.sync.dma_start(out=xh[kk], in_=xTh[ks, hs])
                        nc.sync.dma_start(out=xl[kk], in_=xTl[ks, hs])
                    if c == 0:
                        load_late_weights()
                        load_late_consts()
                kb = slice(128 * k, 128 * (k + 1))
                z = zpool.tile([128, NODES], F32, tag="z")
                # z = xh@wh + xh@wl + xl@wh  (xl@wl term negligible)
                pair = 0
                for kk in range(2):
                    for lhs, rhs in (
                        (xh[kk], wh[kk]),
                        (xh[kk], wl[kk]),
                        (xl[kk], whb[kk]),
                    ):
                        for nh in range(2):
                            ns = slice(512 * nh, 512 * (nh + 1))
                            nc.tensor.matmul(
                                z[:, ns],
                                lhs[:, kb],
                                rhs[:, ns],
                                start=(pair == 0),
                                stop=(pair == 5),
                            )
                        pair += 1
                # u = (z < -b_pred); contiguous [128, 1024] write.
                # Explicit DVE/ACT split: ACT eviction uses the saturated
                # sigmoid trick u = Sigmoid(-1e30 * z) which is exactly
                # {0, 1} fp for any |z| > 1e-28.
                if use_sign_path:
                    if k % 8 < 4:
                        nc.scalar.activation(
                            out=s_chunk[:, k, :],
                            in_=z[:, :],
                            func=mybir.ActivationFunctionType.Sigmoid,
                            scale=-1e30,
                        )
                    else:
                        nc.vector.tensor_scalar(
                            out=s_chunk[:, k, :],
                            in0=z[:, :],
                            scalar1=0.0,
                            scalar2=None,
                            op0=mybir.AluOpType.is_lt,
                        )
                else:
                    nc.vector.tensor_tensor(
                        out=s_chunk[:, k, :],
                        in0=z[:, :],
                        in1=th[:, :],
                        op=mybir.AluOpType.is_lt,
                    )

            # ---- bottom-up collapse (fp16; all WRITES contiguous) ----
            # r_9 = u at level-9 nodes (columns 511..1022)
            r_prev = s_chunk[:, :, 511:1023]
            for i in range(8, -1, -1):
                n = 1 << i
                kconst = float(1 << (9 - i))
                u_i = s_chunk[:, :, n - 1 : 2 * n - 1]
                rp = r_prev.rearrange("p b (n two) -> p b n two", two=2)
                r_e = rp[:, :, :, 0]
                r_o = rp[:, :, :, 1]
                d_t = dpool.tile([128, NB, n], FP16, tag="d")
                # D = (r_o + K) - r_e
                nc.vector.scalar_tensor_tensor(
                    out=d_t,
                    in0=r_o,
                    scalar=kconst,
                    in1=r_e,
                    op0=mybir.AluOpType.add,
                    op1=mybir.AluOpType.subtract,
                )
                # D *= u
                nc.vector.tensor_tensor(
                    out=d_t, in0=u_i, in1=d_t, op=mybir.AluOpType.mult
                )
                # r = r_e + D
                if i > 0:
                    r_t = rpool.tile([128, NB, n], FP16, tag="r")
                    nc.vector.tensor_tensor(
                        out=r_t, in0=r_e, in1=d_t, op=mybir.AluOpType.add
                    )
                    r_prev = r_t
                else:
                    nc.vector.tensor_tensor(
                        out=leaf_all[:, c * NB : (c + 1) * NB],
                        in0=r_e[:, :, 0],
                        in1=d_t[:, :, 0],
                        op=mybir.AluOpType.add,
                    )

            # ---- per-chunk output stage ----
            # leaf -> int16. Sample rows are host-permuted within each
            # 128-block by pi(p) = 8*(p%16) + p//16, so leaf_i16 is already
            # in ap_gather's wrapped index layout and outputs land in
            # natural row order.
            cslice = slice(NB * c, NB * (c + 1))
            nc.vector.tensor_copy(
                out=leaf_i16[:, cslice], in_=leaf_all[:, cslice]
            )
            # table gathers: R[16g+cls, j] = T[cls, leaf(sample 8j+g)]
            rs = slice(128 * c, 128 * (c + 1))
            for tbl, rbuf in ((t_out, r_out), (t_std, r_std)):
                nc.gpsimd.ap_gather(
                    out_ap=rbuf[:, rs],
                    in_ap=tbl,
                    idxs_ap=leaf_i16[:, cslice],
                    channels=128,
                    num_elems=NODES,
                    d=1,
                    num_idxs=128,
                )
            if c >= LAG:
                emit_out_chain(c - LAG)

        for c in range(NCH - LAG, NCH):
            emit_out_chain(c)

    nc.compile()
    return nc


_CACHE = {}


def _get_nc(use_sign_path: bool):
    key = use_sign_path
    if key not in _CACHE:
        nc = bacc.Bacc("TRN2", target_bir_lowering=False, debug=False)
        _CACHE[key] = _build(nc, use_sign_path)
    return _CACHE[key]


# Within each 128-row block, device partition p holds sample row PERM[p].
# PERM aligns the collapse output with ap_gather's wrapped index layout and
# makes the final outputs land in natural row order (see kernel() docstring).
PERM = np.array([8 * (p % 16) + p // 16 for p in range(128)], dtype=np.int64)


def _e8m11(x):
    """Round fp32 to the HW fp32r format (8-bit exp, 11-bit mantissa, RNE)."""
    u = np.ascontiguousarray(x, np.float32).view(np.uint32)
    low = u & np.uint32(0xFFF)
    base = u & np.uint32(0xFFFFF000)
    add = (low > 0x800) | ((low == 0x800) & ((u >> 12) & 1).astype(bool))
    return (base + np.where(add, np.uint32(0x1000), np.uint32(0))).view(np.float32)


def _split_hi_lo(a, lo_bf16=False):
    hi = _e8m11(a)
    lo = (a - hi).astype(np.float32)  # exactly e8m11-representable
    if lo_bf16:
        import ml_dtypes
        lo = lo.astype(ml_dtypes.bfloat16)
    return hi, lo


def _shard_xT(x_shard):
    """[8192, 256] sample rows -> permuted, transposed [256, 8192] device input."""
    xp = x_shard.reshape(NT, 128, IN_DIM)[:, PERM, :].reshape(B_LOC, IN_DIM)
    return np.ascontiguousarray(xp.T)


def _prepare(x, W_pred, b_pred, W_or, action_stds):
    x = np.ascontiguousarray(x, dtype=np.float32)
    W_pred = np.asarray(W_pred, dtype=np.float32)
    b_pred = np.asarray(b_pred, dtype=np.float32)
    W_or = np.asarray(W_or, dtype=np.float32)
    action_stds = np.asarray(action_stds, dtype=np.float32)

    n_int = 2**HEIGHT - 1
    Wp = np.zeros((IN_DIM, NODES), np.float32)
    Wp[:, :n_int] = W_pred.T
    Wph, Wpl = _split_hi_lo(Wp)
    import ml_dtypes
    Wpb = Wph.astype(ml_dtypes.bfloat16)
    # softmax over classes per leaf column
    m = W_or.max(axis=0, keepdims=True)
    e = np.exp(W_or - m)
    t_out16 = (e / e.sum(axis=0, keepdims=True)).astype(np.float32)  # [16, 1024]
    t_std16 = np.clip(action_stds, -20.0, 2.0).astype(np.float32)
    t_out = np.tile(t_out16, (8, 1))  # [128, 1024]
    t_std = np.tile(t_std16, (8, 1))
    th16 = np.zeros((NODES,), np.float32)
    th16[:n_int] = -b_pred
    th = np.tile(th16[None, :], (128, 1))
    return x, Wph, Wpl, Wpb, t_out, t_std, th, bool(np.any(b_pred != 0.0))


def kernel(x, W_pred, b_pred, W_or, action_stds, _want_trace=False):
    x, Wph, Wpl, Wpb, t_out, t_std, th, b_nonzero = _prepare(
        x, W_pred, b_pred, W_or, action_stds
    )
    nc = _get_nc(use_sign_path=not b_nonzero)

    in_maps = []
    for c in range(N_CORES):
        shard = x[c * B_LOC : (c + 1) * B_LOC]
        xt = _shard_xT(shard)
        xth, xtl = _split_hi_lo(xt, lo_bf16=True)
        in_maps.append(
            {
                "xTh": xth,
                "xTl": xtl,
                "Wph": Wph,
                "Wpl": Wpl,
                "Wpb": Wpb,
                "Tout": t_out,
                "Tstd": t_std,
                "TH": th,
                "Ident": np.eye(128, dtype=np.float32),
            }
        )

    res = run_bass_kernel_spmd(
        nc, in_maps, core_ids=list(range(N_CORES)), trace=_want_trace
    )
    out = np.concatenate([res.results[c]["out_o"] for c in range(N_CORES)], axis=0)
    std = np.concatenate([res.results[c]["out_s"] for c in range(N_CORES)], axis=0)
    if _want_trace:
        kernel.last_results = res
    return out, std


# revision 26
# speedup vs baseline: 2.2094x; 1.0040x over previous
"""Trainium2 Bass kernel for nn_DGT_6485400616966 (soft decision tree forward).

Math (forward pass only):
  pred_z = x @ W_pred.T + b_pred                      [B, 1023]
  The straight-through/one-hot structure collapses: the output depends only on
  the argmax leaf of the tree AND layer, which equals a 10-level tree descent
  following sign(pred_z) at visited nodes (left if z >= 0).
  out = softmax(W_or[:, leaf]) ; std = clip(action_stds[:, leaf], -20, 2)

Device algorithm per core (8192 samples, data-parallel over 8 cores):
  1. PE: z = x @ W_pred.T in three fp32r passes (xh@wh + xh@wl + xl@wh) where
     hi/lo are an exact e8m11 split of the fp32 operands (fp32r on HW is
     e8m11; one pass alone flips ~38 argmax rows, three passes flip none).
     x tiles are the stationary operand; W^T columns (nodes, padded to 1024)
     are the moving operand, N=512 per matmul for full fp32r rate.
  2. Eviction PSUM->SBUF per btile: u = (z < 0) as fp16, contiguous writes
     (strided 2-byte DVE writes cost ~4x). Split DVE tensor_scalar is_lt /
     ACT saturated-sigmoid (Sigmoid(-1e30*z) is exactly {0,1}).
  3. DVE: bottom-up tree collapse r_i = r_e + u_i*(K + r_o - r_e) in fp16 on
     [128, NB, 2^i] chunk tensors (btile-major; all writes contiguous).
  4. GPSIMD ap_gather per chunk: table lookup T[class, leaf] with the 16
     classes replicated on partitions; each 16-partition group shares its
     sample's leaf index (host pre-permutes rows by pi(p)=8*(p%16)+p//16 so
     indices are already wrapped and outputs land in natural order).
  5. PE transpose of the gathered [128, 128] blocks (emitted LAG chunks late
     so the in-order PE queue never stalls) + contiguous DMA out.
"""

import sys

for _p in ("/opt/trn_rl_repo",):
    if _p not in sys.path:
        sys.path.insert(0, _p)

from contextlib import ExitStack

import numpy as np

import concourse.bacc as bacc
import concourse.bass as bass
import concourse.tile as tile
from concourse import mybir
from concourse.bass_utils import run_bass_kernel_spmd

HEIGHT = 10
IN_DIM = 256
OUT_DIM = 16
BATCH = 65536
N_CORES = 8
B_LOC = BATCH // N_CORES          # 8192 samples per core
NT = B_LOC // 128                 # 64 batch tiles of 128 samples
NB = 8                            # btiles per collapse chunk
NCH = NT // NB                    # 8 chunks
NODES = 1024                      # 1023 real + 1 pad
F32 = mybir.dt.float32
F32R = mybir.dt.float32r
BF16 = mybir.dt.bfloat16
FP16 = mybir.dt.float16
I16 = mybir.dt.int16


def _build(nc, use_sign_path: bool):
    """Emit the per-core program. use_sign_path=True assumes b_pred == 0."""
    # hi/lo e8m11 split operands (fp32r is e8m11 on HW; hi+lo == fp32 exactly)
    xTh = nc.dram_tensor("xTh", [IN_DIM, B_LOC], F32R, kind="ExternalInput")
    xTl = nc.dram_tensor("xTl", [IN_DIM, B_LOC], BF16, kind="ExternalInput")
    Wph = nc.dram_tensor("Wph", [IN_DIM, NODES], F32R, kind="ExternalInput")
    Wpl = nc.dram_tensor("Wpl", [IN_DIM, NODES], F32R, kind="ExternalInput")
    Wpb = nc.dram_tensor("Wpb", [IN_DIM, NODES], BF16, kind="ExternalInput")
    Tout = nc.dram_tensor("Tout", [128, NODES], F32, kind="ExternalInput")
    Tstd = nc.dram_tensor("Tstd", [128, NODES], F32, kind="ExternalInput")
    TH = nc.dram_tensor("TH", [128, NODES], F32, kind="ExternalInput")
    Ident = nc.dram_tensor("Ident", [128, 128], F32, kind="ExternalInput")
    out_o = nc.dram_tensor("out_o", [B_LOC, OUT_DIM], F32, kind="ExternalOutput")
    out_s = nc.dram_tensor("out_s", [B_LOC, OUT_DIM], F32, kind="ExternalOutput")

    with tile.TileContext(nc) as tc, ExitStack() as ctx:
        consts = ctx.enter_context(tc.tile_pool(name="consts", bufs=1))
        xpool = ctx.enter_context(tc.tile_pool(name="xpool", bufs=4))
        spool = ctx.enter_context(tc.tile_pool(name="spool", bufs=3))
        rpool = ctx.enter_context(tc.tile_pool(name="rpool", bufs=3))
        dpool = ctx.enter_context(tc.tile_pool(name="dpool", bufs=3))
        zpool = ctx.enter_context(
            tc.tile_pool(name="zpool", bufs=3, space=bass.MemorySpace.PSUM)
        )
        tpool = ctx.enter_context(
            tc.tile_pool(name="tpool", bufs=2, space=bass.MemorySpace.PSUM)
        )

        wh = [
            consts.tile([128, NODES], F32R, tag=f"wh{k}", name=f"wh{k}")
            for k in range(2)
        ]
        wl = [
            consts.tile([128, NODES], F32R, tag=f"wl{k}", name=f"wl{k}")
            for k in range(2)
        ]
        whb = [
            consts.tile([128, NODES], BF16, tag=f"whb{k}", name=f"whb{k}")
            for k in range(2)
        ]
        nc.sync.dma_start(out=wh[0], in_=Wph[0:128, :])

        def load_late_weights():
            nc.sync.dma_start(out=whb[0], in_=Wpb[0:128, :])
            nc.sync.dma_start(out=wl[0], in_=Wpl[0:128, :])
            nc.sync.dma_start(out=wh[1], in_=Wph[128:256, :])
            nc.sync.dma_start(out=whb[1], in_=Wpb[128:256, :])
            nc.sync.dma_start(out=wl[1], in_=Wpl[128:256, :])
        t_out = consts.tile([128, NODES], F32)
        t_std = consts.tile([128, NODES], F32)
        ident = consts.tile([128, 128], F32)
        th = None
        if not use_sign_path:
            th = consts.tile([128, NODES], F32)
            nc.sync.dma_start(out=th, in_=TH[:, :])

        def load_late_consts():
            # tables/identity are first consumed by the descent/output stage;
            # loading them after the first chunk's x keeps the PE start early.
            nc.sync.dma_start(out=t_out, in_=Tout[:, :])
            nc.sync.dma_start(out=t_std, in_=Tstd[:, :])
            nc.sync.dma_start(out=ident, in_=Ident[:, :])

        leaf_all = consts.tile([128, NT], FP16)
        leaf_i16 = consts.tile([128, NT], I16)
        r_out = consts.tile([128, NODES], F32)
        r_std = consts.tile([128, NODES], F32)

        o_view = out_o.rearrange("(t p f) c -> t p (f c)", t=8, p=128, f=8)
        s_view = out_s.rearrange("(t p f) c -> t p (f c)", t=8, p=128, f=8)
        LAG = 2

        def emit_out_chain(cc):
            # transpose chunk cc's gathered [128, 128] table blocks and DMA
            # them out; emitted LAG chunks late so the in-order PE queue
            # never stalls on the descent chain.
            rs_ = slice(128 * cc, 128 * (cc + 1))
            for rbuf, dview in ((r_out, o_view), (r_std, s_view)):
                pt = tpool.tile([128, 128], F32, tag="t", name="pt")
                nc.tensor.transpose(pt, rbuf[:, rs_], ident)
                rt = xpool.tile([128, 128], F32, tag="rt", name="rt", bufs=2)
                nc.scalar.copy(out=rt, in_=pt)
                nc.sync.dma_start(out=dview[cc], in_=rt)

        for c in range(NCH):
            # btile-MAJOR u-bit store: eviction writes [128, 1024] contiguous
            # (strided 2-byte writes cost ~4x on DVE; reads don't).
            s_chunk = spool.tile([128, NB, NODES], FP16, tag="s")
            for k in range(NB):
                t = c * NB + k
                bs = slice(128 * t, 128 * (t + 1))
                if k == 0:
                    # stage x for this chunk: [128, 128*NB] per ktile/half
                    hs = slice(128 * NB * c, 128 * NB * (c + 1))
                    xh = [
                        xpool.tile(
                            [128, 128 * NB], F32R,
                            tag=f"xh{kk}", name=f"xh{kk}", bufs=2,
                        )
                        for kk in range(2)
                    ]
                    xl = [
                        xpool.tile(
                            [128, 128 * NB], BF16,
                            tag=f"xl{kk}", name=f"xl{kk}", bufs=2,
                        )
                        for kk in range(2)
                    ]
                    for kk in range(2):
                        ks = slice(128 * kk, 128 * (kk + 1))
                        nc.sync.dma_start(out=xh[kk], in_=xTh[ks, hs])
                        nc.sync.dma_start(out=xl[kk], in_=xTl[ks, hs])
                        if c == 0 and kk == 0:
                            load_late_weights()
                    if c == 0:
                        load_late_consts()
                kb = slice(128 * k, 128 * (k + 1))
                z = zpool.tile([128, NODES], F32, tag="z")
                # z = xh@wh + xh@wl + xl@wh  (xl@wl term negligible)
                pair = 0
                for kk in range(2):
                    for lhs, rhs in (
                        (xh[kk], wh[kk]),
                        (xl[kk], whb[kk]),
                        (xh[kk], wl[kk]),
                    ):
                        for nh in range(2):
                            ns = slice(512 * nh, 512 * (nh + 1))
                            nc.tensor.matmul(
                                z[:, ns],
                                lhs[:, kb],
                                rhs[:, ns],
                                start=(pair == 0),
                                stop=(pair == 5),
                            )
                        pair += 1
                # u = (z < -b_pred); contiguous [128, 1024] write.
                # Explicit DVE/ACT split: ACT eviction uses the saturated
                # sigmoid trick u = Sigmoid(-1e30 * z) which is exactly
                # {0, 1} fp for any |z| > 1e-28.
                if use_sign_path:
                    if k % 8 < 4:
                        nc.scalar.activation(
                            out=s_chunk[:, k, :],
                            in_=z[:, :],
                            func=mybir.ActivationFunctionType.Sigmoid,
                            scale=-1e30,
                        )
                    else:
                        nc.vector.tensor_scalar(
                            out=s_chunk[:, k, :],
                            in0=z[:, :],
                            scalar1=0.0,
                            scalar2=None,
                            op0=mybir.AluOpType.is_lt,
                        )
                else:
                    nc.vector.tensor_tensor(
                        out=s_chunk[:, k, :],
                        in0=z[:, :],
                        in1=th[:, :],
                        op=mybir.AluOpType.is_lt,
                    )

            # ---- bottom-up collapse (fp16; all WRITES contiguous) ----
            # r_9 = u at level-9 nodes (columns 511..1022)
            r_prev = s_chunk[:, :, 511:1023]
            for i in range(8, -1, -1):
                n = 1 << i
                kconst = float(1 << (9 - i))
                u_i = s_chunk[:, :, n - 1 : 2 * n - 1]
                rp = r_prev.rearrange("p b (n two) -> p b n two", two=2)
                r_e = rp[:, :, :, 0]
                r_o = rp[:, :, :, 1]
                d_t = dpool.tile([128, NB, n], FP16, tag="d")
                # D = (r_o + K) - r_e
                nc.vector.scalar_tensor_tensor(
                    out=d_t,
                    in0=r_o,
                    scalar=kconst,
                    in1=r_e,
                    op0=mybir.AluOpType.add,
                    op1=mybir.AluOpType.subtract,
                )
                # D *= u
                nc.vector.tensor_tensor(
                    out=d_t, in0=u_i, in1=d_t, op=mybir.AluOpType.mult
                )
                # r = r_e + D
                if i > 0:
                    r_t = rpool.tile([128, NB, n], FP16, tag="r")
                    nc.vector.tensor_tensor(
                        out=r_t, in0=r_e, in1=d_t, op=mybir.AluOpType.add
                    )
                    r_prev = r_t
                else:
                    nc.vector.tensor_tensor(
                        out=leaf_all[:, c * NB : (c + 1) * NB],
                        in0=r_e[:, :, 0],
                        in1=d_t[:, :, 0],
                        op=mybir.AluOpType.add,
                    )

            # ---- per-chunk output stage ----
            # leaf -> int16. Sample rows are host-permuted within each
            # 128-block by pi(p) = 8*(p%16) + p//16, so leaf_i16 is already
            # in ap_gather's wrapped index layout and outputs land in
            # natural row order.
            cslice = slice(NB * c, NB * (c + 1))
            nc.vector.tensor_copy(
                out=leaf_i16[:, cslice], in_=leaf_all[:, cslice]
            )
            # table gathers: R[16g+cls, j] = T[cls, leaf(sample 8j+g)]
            rs = slice(128 * c, 128 * (c + 1))
            for tbl, rbuf in ((t_out, r_out), (t_std, r_std)):
                nc.gpsimd.ap_gather(
                    out_ap=rbuf[:, rs],
                    in_ap=tbl,
                    idxs_ap=leaf_i16[:, cslice],
                    channels=128,
                    num_elems=NODES,
                    d=1,
                    num_idxs=128,
                )
            if c >= LAG:
                emit_out_chain(c - LAG)

        for c in range(NCH - LAG, NCH):
            emit_out_chain(c)

    nc.compile()
    return nc


_CACHE = {}


def _get_nc(use_sign_path: bool):
    key = use_sign_path
    if key not in _CACHE:
        nc = bacc.Bacc("TRN2", target_bir_lowering=False, debug=False)
        _CACHE[key] = _build(nc, use_sign_path)
    return _CACHE[key]


# Within each 128-row block, device partition p holds sample row PERM[p].
# PERM aligns the collapse output with ap_gather's wrapped index layout and
# makes the final outputs land in natural row order (see kernel() docstring).
PERM = np.array([8 * (p % 16) + p // 16 for p in range(128)], dtype=np.int64)


def _e8m11(x):
    """Round fp32 to the HW fp32r format (8-bit exp, 11-bit mantissa, RNE)."""
    u = np.ascontiguousarray(x, np.float32).view(np.uint32)
    low = u & np.uint32(0xFFF)
    base = u & np.uint32(0xFFFFF000)
    add = (low > 0x800) | ((low == 0x800) & ((u >> 12) & 1).astype(bool))
    return (base + np.where(add, np.uint32(0x1000), np.uint32(0))).view(np.float32)


def _split_hi_lo(a, lo_bf16=False):
    hi = _e8m11(a)
    lo = (a - hi).astype(np.float32)  # exactly e8m11-representable
    if lo_bf16:
        import ml_dtypes
        lo = lo.astype(ml_dtypes.bfloat16)
    return hi, lo


def _shard_xT(x_shard):
    """[8192, 256] sample rows -> permuted, transposed [256, 8192] device input."""
    xp = x_shard.reshape(NT, 128, IN_DIM)[:, PERM, :].reshape(B_LOC, IN_DIM)
    return np.ascontiguousarray(xp.T)


def _prepare(x, W_pred, b_pred, W_or, action_stds):
    x = np.ascontiguousarray(x, dtype=np.float32)
    W_pred = np.asarray(W_pred, dtype=np.float32)
    b_pred = np.asarray(b_pred, dtype=np.float32)
    W_or = np.asarray(W_or, dtype=np.float32)
    action_stds = np.asarray(action_stds, dtype=np.float32)

    n_int = 2**HEIGHT - 1
    Wp = np.zeros((IN_DIM, NODES), np.float32)
    Wp[:, :n_int] = W_pred.T
    Wph, Wpl = _split_hi_lo(Wp)
    import ml_dtypes
    Wpb = Wph.astype(ml_dtypes.bfloat16)
    # softmax over classes per leaf column
    m = W_or.max(axis=0, keepdims=True)
    e = np.exp(W_or - m)
    t_out16 = (e / e.sum(axis=0, keepdims=True)).astype(np.float32)  # [16, 1024]
    t_std16 = np.clip(action_stds, -20.0, 2.0).astype(np.float32)
    t_out = np.tile(t_out16, (8, 1))  # [128, 1024]
    t_std = np.tile(t_std16, (8, 1))
    th16 = np.zeros((NODES,), np.float32)
    th16[:n_int] = -b_pred
    th = np.tile(th16[None, :], (128, 1))
    return x, Wph, Wpl, Wpb, t_out, t_std, th, bool(np.any(b_pred != 0.0))


def kernel(x, W_pred, b_pred, W_or, action_stds, _want_trace=False):
    x, Wph, Wpl, Wpb, t_out, t_std, th, b_nonzero = _prepare(
        x, W_pred, b_pred, W_or, action_stds
    )
    nc = _get_nc(use_sign_path=not b_nonzero)

    in_maps = []
    for c in range(N_CORES):
        shard = x[c * B_LOC : (c + 1) * B_LOC]
        xt = _shard_xT(shard)
        xth, xtl = _split_hi_lo(xt, lo_bf16=True)
        in_maps.append(
            {
                "xTh": xth,
                "xTl": xtl,
                "Wph": Wph,
                "Wpl": Wpl,
                "Wpb": Wpb,
                "Tout": t_out,
                "Tstd": t_std,
                "TH": th,
                "Ident": np.eye(128, dtype=np.float32),
            }
        )

    res = run_bass_kernel_spmd(
        nc, in_maps, core_ids=list(range(N_CORES)), trace=_want_trace
    )
    out = np.concatenate([res.results[c]["out_o"] for c in range(N_CORES)], axis=0)
    std = np.concatenate([res.results[c]["out_s"] for c in range(N_CORES)], axis=0)
    if _want_trace:
        kernel.last_results = res
    return out, std
